# revision 8
# baseline (speedup 1.0000x reference)
"""Multi-head attention (bs=4, seq=2048, hidden=1024, 16 heads) on 8 trn2 cores.

Sharding: core = (batch b, head-group g): 4 batches x 2 groups of 8 heads.
Each core computes QKV projections for its head slice, causal+padded softmax
attention, and a partial output projection; the host sums the two partial
outputs per batch and adds o_b (+ the V-bias contribution, constant across
queries because attention weights sum to 1). K-bias is dropped entirely
(softmax shift invariance).

Engine plan (per core):
  TensorE: QK projections (fp32r), V projection (bf16, FWL), scores
    kT.T@qT transposed [k, q] (fp32r), AV with augmented-V (ones column ->
    softmax denominators accumulate in PSUM row 64), output projection bf16.
  ACT: exclusively exp (padding mask as per-partition bias).
  DVE: projection drains (+q bias), causal tri-mask on bf16 et, softmax
    normalize muls, PSUM->SBUF copies.
  Pool: denominator row broadcast. DMA: den-row partition move, all loads.
Attention is software-pipelined per head (scores of chunk c+1 issue before
AV of chunk c so TensorE never idles on ACT); V1 projections fill window-0
attention, w0 output projections fill window-1 attention.
"""
import os
import sys

for _p in ("/opt/trn_rl_repo",):
    if _p not in sys.path:
        sys.path.insert(0, _p)

import numpy as np

HID = 1024
HEADS = 16
D = 64
BS = 4
SEQ = 2048
NCORES = 8
HG = 2             # head groups (tensor-parallel axis)
HPG = HEADS // HG  # 8 heads per core
OG = HPG * D       # 512 projection dims per core
KC = HID // 128    # 8 hidden chunks
SC = SEQ // 128    # 16 seq chunks
W = 1024           # attention query window
SCALE = 1.0 / np.sqrt(D)
NEG = -30000.0

_compiled = None


def _build():
    import concourse.tile as tile
    from concourse import bacc, mybir

    F32 = mybir.dt.float32
    F32R = mybir.dt.float32r
    BF16 = mybir.dt.bfloat16
    AF = mybir.ActivationFunctionType
    Alu = mybir.AluOpType

    nc = bacc.Bacc("TRN2", target_bir_lowering=False, debug=False,
                   num_devices=NCORES)

    xT_d = nc.dram_tensor("xT", [HID, SEQ], F32R, kind="ExternalInput").ap()
    xTb_d = nc.dram_tensor("xTb", [HID, SEQ], BF16, kind="ExternalInput").ap()
    wqT_d = nc.dram_tensor("wqT", [HID, OG], F32R, kind="ExternalInput").ap()
    wkT_d = nc.dram_tensor("wkT", [HID, OG], F32R, kind="ExternalInput").ap()
    wvT_d = nc.dram_tensor("wvT", [HID, OG], BF16, kind="ExternalInput").ap()
    woT_d = nc.dram_tensor("woT", [OG, HID], BF16, kind="ExternalInput").ap()
    qb_d = nc.dram_tensor("qb", [128, 4], F32, kind="ExternalInput").ap()
    kmask_d = nc.dram_tensor("kmask", [128, SC], F32, kind="ExternalInput").ap()
    out_d = nc.dram_tensor("out", [SEQ, HID], F32, kind="ExternalOutput").ap()

    with tile.TileContext(nc) as tc:
        with tc.tile_pool(name="const", bufs=1) as cp, \
             tc.tile_pool(name="qT", bufs=1) as qTp, \
             tc.tile_pool(name="kT", bufs=1) as kTp, \
             tc.tile_pool(name="v", bufs=1) as vp, \
             tc.tile_pool(name="attnT", bufs=1) as aTp, \
             tc.tile_pool(name="wv", bufs=1) as wvp, \
             tc.tile_pool(name="xv", bufs=1) as xvp:

            # ---------------- constants ----------------
            ones_f = cp.tile([128, 128], F32, tag="ones_f", name="ones_f")
            nc.gpsimd.memset(ones_f[:, :], 1.0)
            # tri01[p, j] = 1 if j >= p else 0  (keep keys <= query)
            tri01_f = cp.tile([128, 128], F32, tag="tri01_f", name="tri01_f")
            nc.gpsimd.affine_select(tri01_f[:, :], ones_f[:, :],
                                    pattern=[[1, 128]],
                                    compare_op=Alu.is_ge, fill=0.0,
                                    base=0, channel_multiplier=-1)
            tri01 = cp.tile([128, 128], BF16, tag="tri01", name="tri01")
            nc.scalar.copy(tri01[:, :], tri01_f[:, :])
            qb_s = cp.tile([128, 4], F32, tag="qb", name="qb_s")
            nc.sync.dma_start(qb_s[:, :], qb_d[:, :])
            kmask_s = cp.tile([128, SC], F32, tag="km", name="kmask_s")
            nc.sync.dma_start(kmask_s[:, :], kmask_d[:, :])
            # pre-warm the Q7 library for partition_broadcast (first use
            # otherwise costs a ~7us LIBRARY_RELOAD mid-attention)
            bwarm = cp.tile([64, 128], F32, tag="bwarm", name="bwarm")
            nc.gpsimd.partition_broadcast(bwarm[0:64, :], ones_f[0:1, :])

            # ---------------- persistent tensors ----------------
            qT_t = [qTp.tile([128, SEQ], BF16, tag=f"qT{i}", name=f"qT{i}")
                    for i in range(4)]
            kT_t = [kTp.tile([128, SEQ], BF16, tag=f"kT{i}", name=f"kT{i}")
                    for i in range(4)]
            v_t = [vp.tile([128, HPG * 65], BF16, tag=f"v{i}", name=f"v{i}")
                   for i in range(SC)]
            for i in range(SC):
                vv = v_t[i].rearrange("p (h c) -> p h c", c=65)
                nc.gpsimd.memset(vv[:, :, 64:65], 1.0)
            attnT_t = [aTp.tile([128, SEQ], BF16, tag=f"aT{i}", name=f"aT{i}")
                       for i in range(4)]

            # =========== region 1: QK projections (all seq) + V ===========
            with tc.tile_pool(name="wqk", bufs=1) as wp, \
                 tc.tile_pool(name="x", bufs=1) as xp, \
                 tc.tile_pool(name="phA", bufs=1, space="PSUM") as phA:

                wq_t, wk_t, wv_t = [], [], []
                for kc in range(KC):
                    wq = wp.tile([128, OG], F32R, tag=f"wq{kc}",
                                 name=f"wq{kc}")
                    nc.sync.dma_start(wq[:, :],
                                      wqT_d[kc * 128:(kc + 1) * 128, :])
                    wq_t.append(wq)
                for kc in range(KC):
                    wk = wp.tile([128, OG], F32R, tag=f"wk{kc}",
                                 name=f"wk{kc}")
                    nc.sync.dma_start(wk[:, :],
                                      wkT_d[kc * 128:(kc + 1) * 128, :])
                    wk_t.append(wk)
                for kc in range(KC):
                    wv = wvp.tile([128, OG], BF16, tag=f"wv{kc}",
                                  name=f"wv{kc}")
                    nc.sync.dma_start(wv[:, :],
                                      wvT_d[kc * 128:(kc + 1) * 128, :])
                    wv_t.append(wv)

                def x_half_load(half):
                    tiles = []
                    for kc in range(KC):
                        xt = xp.tile([128, W], F32R, tag=f"x{kc}", bufs=1,
                                     name=f"x{half}{kc}")
                        nc.sync.dma_start(
                            xt[:, :], xT_d[kc * 128:(kc + 1) * 128,
                                           half * W:(half + 1) * W])
                        tiles.append(xt)
                    return tiles

                def qk_unit(wt, oc, xg, half, is_q):
                    p0 = phA.tile([128, 512], F32, tag="pts", bufs=6,
                                  name="p0")
                    p1 = phA.tile([128, 512], F32, tag="pts", bufs=6,
                                  name="p1")
                    for kc in range(KC):
                        for t, pt in ((0, p0), (1, p1)):
                            nc.tensor.matmul(
                                pt[:, :],
                                wt[kc][:, oc * 128:(oc + 1) * 128],
                                xg[kc][:, t * 512:(t + 1) * 512],
                                start=(kc == 0), stop=(kc == KC - 1))
                    o_t = qT_t if is_q else kT_t
                    for t, pt in ((0, p0), (1, p1)):
                        cols = slice(half * W + t * 512,
                                     half * W + t * 512 + 512)
                        if is_q:
                            nc.vector.tensor_scalar(
                                o_t[oc][:, cols], pt[:, :],
                                qb_s[:, oc:oc + 1], None, Alu.add)
                        else:
                            nc.vector.tensor_copy(o_t[oc][:, cols], pt[:, :])

                def v_unit(sc, pool_ref):
                    """V projection for seq chunk sc; bf16 x streamed.
                    pool_ref: 1-elem list holding (psum_pool, tag) at
                    emission time."""
                    cbs = []
                    st = {"xv": [None] * KC}
                    for kc in range(KC):
                        def dma(kc=kc):
                            xv = xvp.tile([128, 128], BF16, tag=f"xv{kc}",
                                          bufs=2, name=f"xv{kc}")
                            nc.sync.dma_start(
                                xv[:, :], xTb_d[kc * 128:(kc + 1) * 128,
                                                sc * 128:(sc + 1) * 128])
                            st["xv"][kc] = xv
                        cbs.append(dma)

                    def alloc():
                        pool, tag = pool_ref[0]
                        st["pv"] = pool.tile([128, 512], F32, tag=tag,
                                             bufs=2, name="pv")
                    cbs.append(alloc)
                    for kc in range(KC):
                        def mm(kc=kc):
                            nc.tensor.matmul(st["pv"][:, :],
                                             st["xv"][kc][:, :],
                                             wv_t[kc][:, :],
                                             start=(kc == 0),
                                             stop=(kc == KC - 1))
                        cbs.append(mm)

                    def drain():
                        src = st["pv"].rearrange("p (h c) -> p h c", c=64)
                        dst = v_t[sc].rearrange("p (h c) -> p h c", c=65)
                        nc.vector.tensor_copy(dst[:, :, 0:64], src[:, :, :])
                    cbs.append(drain)
                    return cbs

                xg = x_half_load(0)
                for oc in range(4):
                    qk_unit(wq_t, oc, xg, 0, True)
                for oc in range(4):
                    qk_unit(wk_t, oc, xg, 0, False)
                # V for seq chunks 0..7 here; the x half-1 loads hide
                # behind these V matmuls.
                xg = x_half_load(1)
                for sc in range(8):
                    for cb in v_unit(sc, [(phA, "pv")]):
                        cb()
                for oc in range(4):
                    qk_unit(wq_t, oc, xg, 1, True)
                for oc in range(4):
                    qk_unit(wk_t, oc, xg, 1, False)

                v1_pool_ref = [None]
                v1_units = [v_unit(sc, v1_pool_ref) for sc in range(8, SC)]

            # ============ region 2: attention + output projection ============
            with tc.tile_pool(name="et", bufs=1) as etp, \
                 tc.tile_pool(name="raw", bufs=1) as rawp, \
                 tc.tile_pool(name="nrm", bufs=1) as nrmp, \
                 tc.tile_pool(name="wo", bufs=1) as wop, \
                 tc.tile_pool(name="ot", bufs=1) as otp, \
                 tc.tile_pool(name="sp", bufs=1, space="PSUM") as spp, \
                 tc.tile_pool(name="at", bufs=1, space="PSUM") as atp, \
                 tc.tile_pool(name="po", bufs=1, space="PSUM") as pop:

                den0 = nrmp.tile([1, W], F32, tag="den0", name="den0")
                denr = nrmp.tile([1, W], F32, tag="denr", name="denr")
                div = nrmp.tile([64, W], F32, tag="div", name="div")

                def attn_head(h, tw, fillers, chunks_left, defer_to,
                              split_nrm=False):
                    hb = (h % 2) * 64
                    hc = h // 2
                    chunks = [(c, 0, W) for c in range(8 * tw)]
                    chunks += [(8 * tw + i, 128 * i, W - 128 * i)
                               for i in range(8)]
                    first = [None, None]
                    last = [None, None]
                    for idx, (c, off, w) in enumerate(chunks):
                        for half in range(2):
                            if max(off, half * 512) < (half + 1) * 512:
                                if first[half] is None:
                                    first[half] = idx
                                last[half] = idx
                    at = atp.tile([65, W], F32, tag="at", bufs=1, name="at")

                    def escore(idx, c, off, w):
                        sp = spp.tile([128, W], F32, tag="sp", bufs=2,
                                      name="sp")
                        for half in range(2):
                            lo = max(off, half * 512)
                            hi = (half + 1) * 512
                            if lo >= hi:
                                continue
                            nc.tensor.matmul(
                                sp[:, lo:hi],
                                kT_t[hc][hb:hb + 64, c * 128:(c + 1) * 128],
                                qT_t[hc][hb:hb + 64,
                                         tw * W + lo:tw * W + hi],
                                start=True, stop=True)
                        return sp

                    def eav(idx, c, off, w, sp):
                        et = etp.tile([128, W], BF16, tag="et", bufs=2,
                                      name="et")
                        nc.scalar.activation(et[:, :w], sp[:, off:off + w],
                                             AF.Exp,
                                             bias=kmask_s[:, c:c + 1],
                                             scale=SCALE)
                        if off or c == 8 * tw:  # diagonal chunk
                            nc.vector.tensor_mul(et[:, 0:128], et[:, 0:128],
                                                 tri01[:, :])
                        for half in range(2):
                            lo = max(off, half * 512)
                            hi = (half + 1) * 512
                            if lo >= hi:
                                continue
                            nc.tensor.matmul(
                                at[0:65, lo:hi],
                                v_t[c][:, h * 65:(h + 1) * 65],
                                et[:, lo - off:hi - off],
                                start=(idx == first[half]),
                                stop=(idx == last[half]))

                    rawat = rawp.tile([65, W], F32, tag="raw", bufs=2,
                                      name="raw")

                    def nrm_half(half):
                        cs = slice(half * 512, (half + 1) * 512)
                        nc.vector.tensor_copy(rawat[0:65, cs], at[0:65, cs])
                        nc.sync.dma_start(den0[0:1, cs], rawat[64:65, cs])
                        nc.vector.reciprocal_approx_fast(denr[0:1, cs],
                                                         den0[0:1, cs])
                        nc.gpsimd.partition_broadcast(div[0:64, cs],
                                                      denr[0:1, cs])
                        nc.vector.tensor_mul(
                            attnT_t[hc][hb:hb + 64,
                                        tw * W + half * 512:
                                        tw * W + (half + 1) * 512],
                            rawat[0:64, cs], div[0:64, cs])

                    prev = None
                    for idx, (c, off, w) in enumerate(chunks):
                        sp = escore(idx, c, off, w)
                        if prev is not None:
                            eav(*prev)
                            if split_nrm and prev[0] == last[0]:
                                nrm_half(0)
                        prev = (idx, c, off, w, sp)
                        n = -(-len(fillers) // max(chunks_left[0], 1))
                        for _ in range(min(n, 5)):
                            if fillers:
                                fillers.pop(0)()
                        chunks_left[0] -= 1
                    eav(*prev)
                    if split_nrm:
                        nrm_half(1)
                        return

                    # normalize: PSUM-releasing copy now; the latency-laden
                    # dma+recip+bcast+mul chain is deferred into the next
                    # attention stretch so it never blocks the DVE queue.
                    nc.vector.tensor_copy(rawat[0:65, :], at[0:65, :])

                    def n_dma():
                        nc.sync.dma_start(den0[0:1, :], rawat[64:65, :])

                    def n_recip():
                        nc.vector.reciprocal_approx_fast(denr[0:1, :],
                                                         den0[0:1, :])

                    def n_bcast():
                        nc.gpsimd.partition_broadcast(div[0:64, :],
                                                      denr[0:1, :])

                    def n_mul():
                        nc.vector.tensor_mul(
                            attnT_t[hc][hb:hb + 64, tw * W:(tw + 1) * W],
                            rawat[0:64, :], div[0:64, :])
                    defer_to.extend([n_dma, n_recip, n_bcast, n_mul])

                wo_t = [None] * 4

                def oproj_unit(sc):
                    cbs = []
                    st = {}

                    def alloc():
                        st["ot"] = otp.tile([128, HID], F32, tag="ot",
                                            bufs=2, name="ot")
                    cbs.append(alloc)
                    for n in range(2):
                        def palloc(n=n):
                            st["po"] = pop.tile([128, 512], F32, tag="po",
                                                bufs=2, name="po")
                        cbs.append(palloc)
                        for kc in range(4):
                            def mm(n=n, kc=kc):
                                nc.tensor.matmul(
                                    st["po"][:, :],
                                    attnT_t[kc][:, sc * 128:(sc + 1) * 128],
                                    wo_t[kc][:, n * 512:(n + 1) * 512],
                                    start=(kc == 0), stop=(kc == 3))
                            cbs.append(mm)

                        def drain(n=n):
                            nc.vector.tensor_copy(
                                st["ot"][:, n * 512:(n + 1) * 512],
                                st["po"][:, :])
                        cbs.append(drain)

                    def store():
                        nc.sync.dma_start(
                            out_d[sc * 128:(sc + 1) * 128, :], st["ot"][:, :])
                    cbs.append(store)
                    return cbs

                # window 0: V1 projections (sc 8..15) fill the gaps
                v1_pool_ref[0] = (pop, "po")
                fillers = []
                for cbs in v1_units:
                    fillers.extend(cbs)
                left = [8 * HPG]
                w0_tail_nrm = []
                for h in range(HPG):
                    defer = fillers if h < HPG - 1 else w0_tail_nrm
                    attn_head(h, 0, fillers, left, defer)
                while fillers:
                    fillers.pop(0)()

                # window 1: w0 output projections fill the gaps
                fillers = []
                def ld_wo():
                    for kc in range(4):
                        wo = wop.tile([128, HID], BF16, tag=f"wo{kc}",
                                      name=f"wo{kc}")
                        nc.sync.dma_start(
                            wo[:, :], woT_d[kc * 128:(kc + 1) * 128, :])
                        wo_t[kc] = wo
                fillers.append(ld_wo)
                fillers.extend(w0_tail_nrm)
                for sc in range(8):
                    fillers += oproj_unit(sc)
                left = [16 * HPG]
                for h in range(HPG):
                    defer = fillers if h < HPG - 1 else []
                    attn_head(h, 1, fillers, left, defer,
                              split_nrm=(h == HPG - 1))
                while fillers:
                    fillers.pop(0)()
                for sc in range(8, SC):
                    for cb in oproj_unit(sc):
                        cb()

    nc.compile()
    return nc


def kernel(hidden_states, causal_mask, padding_mask,
           q_w, q_b, k_w, k_b, v_w, v_b, o_w, o_b):
    global _compiled
    import ml_dtypes
    from concourse.bass_utils import run_bass_kernel_spmd

    hidden_states = np.asarray(hidden_states, dtype=np.float32)
    padding_mask = np.asarray(padding_mask)
    q_w = np.asarray(q_w, dtype=np.float32)
    k_w = np.asarray(k_w, dtype=np.float32)
    v_w = np.asarray(v_w, dtype=np.float32)
    o_w = np.asarray(o_w, dtype=np.float32)
    q_b = np.asarray(q_b, dtype=np.float32)
    v_b = np.asarray(v_b, dtype=np.float32)
    o_b = np.asarray(o_b, dtype=np.float32)

    if _compiled is None:
        _compiled = _build()
    nc = _compiled

    in_maps = []
    for b in range(BS):
        xT = np.ascontiguousarray(hidden_states[b].T)
        xTb = xT.astype(ml_dtypes.bfloat16)
        kmask = np.where(padding_mask[b], np.float32(NEG),
                         np.float32(0.0)).astype(np.float32)
        kmask2 = np.ascontiguousarray(kmask.reshape(SC, 128).T)
        for g in range(HG):
            r = slice(g * OG, (g + 1) * OG)
            in_maps.append({
                "xT": xT,
                "xTb": xTb,
                "wqT": np.ascontiguousarray(q_w[r].T),
                "wkT": np.ascontiguousarray(k_w[r].T),
                "wvT": np.ascontiguousarray(v_w[r].T).astype(
                    ml_dtypes.bfloat16),
                "woT": np.ascontiguousarray(o_w[:, r].T).astype(
                    ml_dtypes.bfloat16),
                "qb": np.ascontiguousarray(q_b[r].reshape(4, 128).T),
                "kmask": kmask2,
            })

    trace = os.environ.get("KERNEL_TRACE") == "1"
    res = run_bass_kernel_spmd(nc, in_maps, core_ids=list(range(NCORES)),
                               trace=trace)
    if trace and res.exec_time_ns is not None:
        print(f"HW exec time: {res.exec_time_ns} ns")
        if res.instructions_and_trace:
            print(f"trace: {res.instructions_and_trace[1]}")

    # host: sum head-group partials, add o_b and the V-bias contribution
    vb_term = o_w @ v_b  # [HID]; exact because attention weights sum to 1
    const = (o_b + vb_term)[None, :]
    out = np.empty((BS, SEQ, HID), dtype=np.float32)
    for b in range(BS):
        out[b] = (res.results[2 * b]["out"] + res.results[2 * b + 1]["out"]
                  + const)
    return out


# revision 9
# speedup vs baseline: 1.0881x; 1.0881x over previous
"""Multi-head attention (bs=4, seq=2048, hidden=1024, 16 heads) on 8 trn2 cores.

Sharding: core = (batch b, head-group g): 4 batches x 2 groups of 8 heads.
Each core computes QKV projections for its head slice, causal+padded softmax
attention, and a partial output projection; the host sums the two partial
outputs per batch and adds o_b (+ the V-bias contribution, constant across
queries because attention weights sum to 1). K-bias is dropped entirely
(softmax shift invariance).

Engine plan (per core):
  TensorE: QK projections (fp32r), V projection (bf16, FWL), scores
    kT.T@qT transposed [k, q] (fp32r), AV with augmented-V (ones column ->
    softmax denominators accumulate in PSUM row 64), output projection bf16.
  ACT: exclusively exp (padding mask as per-partition bias).
  DVE: projection drains (+q bias), causal tri-mask on bf16 et, softmax
    normalize muls, PSUM->SBUF copies.
  Pool: denominator row broadcast. DMA: den-row partition move, all loads.
Attention is software-pipelined per head (scores of chunk c+1 issue before
AV of chunk c so TensorE never idles on ACT); V1 projections fill window-0
attention, w0 output projections fill window-1 attention.
"""
import os
import sys

for _p in ("/opt/trn_rl_repo",):
    if _p not in sys.path:
        sys.path.insert(0, _p)

import numpy as np

HID = 1024
HEADS = 16
D = 64
BS = 4
SEQ = 2048
NCORES = 8
HG = 2             # head groups (tensor-parallel axis)
HPG = HEADS // HG  # 8 heads per core
OG = HPG * D       # 512 projection dims per core
KC = HID // 128    # 8 hidden chunks
SC = SEQ // 128    # 16 seq chunks
W = 1024           # attention query window
SCALE = 1.0 / np.sqrt(D)
NEG = -30000.0

_compiled = None


def _build():
    import concourse.tile as tile
    from concourse import bacc, mybir

    F32 = mybir.dt.float32
    F32R = mybir.dt.float32r
    BF16 = mybir.dt.bfloat16
    AF = mybir.ActivationFunctionType
    Alu = mybir.AluOpType

    nc = bacc.Bacc("TRN2", target_bir_lowering=False, debug=False,
                   num_devices=NCORES)

    xT_d = nc.dram_tensor("xT", [HID, SEQ], F32R, kind="ExternalInput").ap()
    xTb_d = nc.dram_tensor("xTb", [HID, SEQ], BF16, kind="ExternalInput").ap()
    wqT_d = nc.dram_tensor("wqT", [HID, OG], F32R, kind="ExternalInput").ap()
    wkT_d = nc.dram_tensor("wkT", [HID, OG], F32R, kind="ExternalInput").ap()
    wvT_d = nc.dram_tensor("wvT", [HID, OG], BF16, kind="ExternalInput").ap()
    woT_d = nc.dram_tensor("woT", [OG, HID], BF16, kind="ExternalInput").ap()
    qb_d = nc.dram_tensor("qb", [128, 4], F32, kind="ExternalInput").ap()
    kmask_d = nc.dram_tensor("kmask", [128, SC], F32, kind="ExternalInput").ap()
    out_d = nc.dram_tensor("out", [SEQ, HID], F32, kind="ExternalOutput").ap()

    with tile.TileContext(nc) as tc:
        with tc.tile_pool(name="const", bufs=1) as cp, \
             tc.tile_pool(name="qT", bufs=1) as qTp, \
             tc.tile_pool(name="kT", bufs=1) as kTp, \
             tc.tile_pool(name="v", bufs=1) as vp, \
             tc.tile_pool(name="attnT", bufs=1) as aTp, \
             tc.tile_pool(name="wv", bufs=1) as wvp, \
             tc.tile_pool(name="xv", bufs=1) as xvp:

            # ---------------- constants ----------------
            ones_f = cp.tile([128, 128], F32, tag="ones_f", name="ones_f")
            nc.gpsimd.memset(ones_f[:, :], 1.0)
            # tri01[p, j] = 1 if j >= p else 0  (keep keys <= query)
            tri01_f = cp.tile([128, 128], F32, tag="tri01_f", name="tri01_f")
            nc.gpsimd.affine_select(tri01_f[:, :], ones_f[:, :],
                                    pattern=[[1, 128]],
                                    compare_op=Alu.is_ge, fill=0.0,
                                    base=0, channel_multiplier=-1)
            tri01 = cp.tile([128, 128], BF16, tag="tri01", name="tri01")
            nc.scalar.copy(tri01[:, :], tri01_f[:, :])
            qb_s = cp.tile([128, 4], F32, tag="qb", name="qb_s")
            nc.sync.dma_start(qb_s[:, :], qb_d[:, :])
            kmask_s = cp.tile([128, SC], F32, tag="km", name="kmask_s")
            nc.sync.dma_start(kmask_s[:, :], kmask_d[:, :])
            # pre-warm the Q7 library for partition_broadcast (first use
            # otherwise costs a ~7us LIBRARY_RELOAD mid-attention)
            bwarm = cp.tile([64, 128], F32, tag="bwarm", name="bwarm")
            nc.gpsimd.partition_broadcast(bwarm[0:64, :], ones_f[0:1, :])

            # ---------------- persistent tensors ----------------
            qT_t = [qTp.tile([128, SEQ], BF16, tag=f"qT{i}", name=f"qT{i}")
                    for i in range(4)]
            kT_t = [kTp.tile([128, SEQ], BF16, tag=f"kT{i}", name=f"kT{i}")
                    for i in range(4)]
            v_t = [vp.tile([128, HPG * 65], BF16, tag=f"v{i}", name=f"v{i}")
                   for i in range(SC)]
            for i in range(SC):
                vv = v_t[i].rearrange("p (h c) -> p h c", c=65)
                nc.gpsimd.memset(vv[:, :, 64:65], 1.0)
            attnT_t = [aTp.tile([128, SEQ], BF16, tag=f"aT{i}", name=f"aT{i}")
                       for i in range(4)]

            # =========== region 1: QK projections (all seq) + V ===========
            with tc.tile_pool(name="wqk", bufs=1) as wp, \
                 tc.tile_pool(name="x", bufs=1) as xp, \
                 tc.tile_pool(name="phA", bufs=1, space="PSUM") as phA:

                wq_t, wk_t, wv_t = [], [], []
                for kc in range(KC):
                    wq = wp.tile([128, OG], F32R, tag=f"wq{kc}",
                                 name=f"wq{kc}")
                    nc.sync.dma_start(wq[:, :],
                                      wqT_d[kc * 128:(kc + 1) * 128, :])
                    wq_t.append(wq)
                for kc in range(KC):
                    wk = wp.tile([128, OG], F32R, tag=f"wk{kc}",
                                 name=f"wk{kc}")
                    nc.sync.dma_start(wk[:, :],
                                      wkT_d[kc * 128:(kc + 1) * 128, :])
                    wk_t.append(wk)
                for kc in range(KC):
                    wv = wvp.tile([128, OG], BF16, tag=f"wv{kc}",
                                  name=f"wv{kc}")
                    nc.sync.dma_start(wv[:, :],
                                      wvT_d[kc * 128:(kc + 1) * 128, :])
                    wv_t.append(wv)

                def x_half_load(half):
                    tiles = []
                    for kc in range(KC):
                        xt = xp.tile([128, W], F32R, tag=f"x{kc}", bufs=1,
                                     name=f"x{half}{kc}")
                        nc.sync.dma_start(
                            xt[:, :], xT_d[kc * 128:(kc + 1) * 128,
                                           half * W:(half + 1) * W])
                        tiles.append(xt)
                    return tiles

                def qk_unit(wt, oc, xg, half, is_q):
                    p0 = phA.tile([128, 512], F32, tag="pts", bufs=6,
                                  name="p0")
                    p1 = phA.tile([128, 512], F32, tag="pts", bufs=6,
                                  name="p1")
                    for kc in range(KC):
                        for t, pt in ((0, p0), (1, p1)):
                            nc.tensor.matmul(
                                pt[:, :],
                                wt[kc][:, oc * 128:(oc + 1) * 128],
                                xg[kc][:, t * 512:(t + 1) * 512],
                                start=(kc == 0), stop=(kc == KC - 1))
                    o_t = qT_t if is_q else kT_t
                    for t, pt in ((0, p0), (1, p1)):
                        cols = slice(half * W + t * 512,
                                     half * W + t * 512 + 512)
                        if is_q:
                            nc.vector.tensor_scalar(
                                o_t[oc][:, cols], pt[:, :],
                                qb_s[:, oc:oc + 1], None, Alu.add)
                        else:
                            nc.vector.tensor_copy(o_t[oc][:, cols], pt[:, :])

                def v_unit(sc, pool_ref):
                    """V projection for seq chunk sc; bf16 x streamed.
                    pool_ref: 1-elem list holding (psum_pool, tag) at
                    emission time."""
                    cbs = []
                    st = {"xv": [None] * KC}
                    for kc in range(KC):
                        def dma(kc=kc):
                            xv = xvp.tile([128, 128], BF16, tag=f"xv{kc}",
                                          bufs=2, name=f"xv{kc}")
                            nc.sync.dma_start(
                                xv[:, :], xTb_d[kc * 128:(kc + 1) * 128,
                                                sc * 128:(sc + 1) * 128])
                            st["xv"][kc] = xv
                        cbs.append(dma)

                    def alloc():
                        pool, tag = pool_ref[0]
                        st["pv"] = pool.tile([128, 512], F32, tag=tag,
                                             bufs=2, name="pv")
                    cbs.append(alloc)
                    for kc in range(KC):
                        def mm(kc=kc):
                            nc.tensor.matmul(st["pv"][:, :],
                                             st["xv"][kc][:, :],
                                             wv_t[kc][:, :],
                                             start=(kc == 0),
                                             stop=(kc == KC - 1))
                        cbs.append(mm)

                    def drain():
                        src = st["pv"].rearrange("p (h c) -> p h c", c=64)
                        dst = v_t[sc].rearrange("p (h c) -> p h c", c=65)
                        nc.vector.tensor_copy(dst[:, :, 0:64], src[:, :, :])
                    cbs.append(drain)
                    return cbs

                xg = x_half_load(0)
                for oc in range(4):
                    qk_unit(wq_t, oc, xg, 0, True)
                for oc in range(4):
                    qk_unit(wk_t, oc, xg, 0, False)
                # V for seq chunks 0..7 here; the x half-1 loads hide
                # behind these V matmuls.
                xg = x_half_load(1)
                for sc in range(8):
                    for cb in v_unit(sc, [(phA, "pv")]):
                        cb()
                for oc in range(4):
                    qk_unit(wq_t, oc, xg, 1, True)
                for oc in range(4):
                    qk_unit(wk_t, oc, xg, 1, False)

                v1_pool_ref = [None]
                v1_units = [v_unit(sc, v1_pool_ref) for sc in range(8, SC)]

            # ============ region 2: attention + output projection ============
            with tc.tile_pool(name="et", bufs=1) as etp, \
                 tc.tile_pool(name="raw", bufs=1) as rawp, \
                 tc.tile_pool(name="nrm", bufs=1) as nrmp, \
                 tc.tile_pool(name="wo", bufs=1) as wop, \
                 tc.tile_pool(name="ot", bufs=1) as otp, \
                 tc.tile_pool(name="sp", bufs=1, space="PSUM") as spp, \
                 tc.tile_pool(name="at", bufs=1, space="PSUM") as atp, \
                 tc.tile_pool(name="po", bufs=1, space="PSUM") as pop:

                den0 = nrmp.tile([1, W], F32, tag="den0", name="den0")
                denr = nrmp.tile([1, W], F32, tag="denr", name="denr")
                div = nrmp.tile([64, W], F32, tag="div", name="div")

                def attn_head(h, tw, fillers, nrm_q, chunks_left, defer_to,
                              split_nrm=False):
                    hb = (h % 2) * 64
                    hc = h // 2
                    chunks = [(c, 0, W) for c in range(8 * tw)]
                    chunks += [(8 * tw + i, 128 * i, W - 128 * i)
                               for i in range(8)]
                    first = [None, None]
                    last = [None, None]
                    for idx, (c, off, w) in enumerate(chunks):
                        for half in range(2):
                            if max(off, half * 512) < (half + 1) * 512:
                                if first[half] is None:
                                    first[half] = idx
                                last[half] = idx
                    at = atp.tile([65, W], F32, tag="at", bufs=1, name="at")

                    def escore(idx, c, off, w):
                        sp = spp.tile([128, W], F32, tag="sp", bufs=2,
                                      name="sp")
                        for half in range(2):
                            lo = max(off, half * 512)
                            hi = (half + 1) * 512
                            if lo >= hi:
                                continue
                            nc.tensor.matmul(
                                sp[:, lo:hi],
                                kT_t[hc][hb:hb + 64, c * 128:(c + 1) * 128],
                                qT_t[hc][hb:hb + 64,
                                         tw * W + lo:tw * W + hi],
                                start=True, stop=True)
                        return sp

                    def eav(idx, c, off, w, sp):
                        et = etp.tile([128, W], BF16, tag="et", bufs=2,
                                      name="et")
                        nc.scalar.activation(et[:, :w], sp[:, off:off + w],
                                             AF.Exp,
                                             bias=kmask_s[:, c:c + 1],
                                             scale=SCALE)
                        if off or c == 8 * tw:  # diagonal chunk
                            nc.vector.tensor_mul(et[:, 0:128], et[:, 0:128],
                                                 tri01[:, :])
                        for half in range(2):
                            lo = max(off, half * 512)
                            hi = (half + 1) * 512
                            if lo >= hi:
                                continue
                            nc.tensor.matmul(
                                at[0:65, lo:hi],
                                v_t[c][:, h * 65:(h + 1) * 65],
                                et[:, lo - off:hi - off],
                                start=(idx == first[half]),
                                stop=(idx == last[half]))

                    rawat = rawp.tile([65, W], F32, tag="raw", bufs=2,
                                      name="raw")

                    def nrm_half(half):
                        cs = slice(half * 512, (half + 1) * 512)
                        nc.vector.tensor_copy(rawat[0:65, cs], at[0:65, cs])
                        nc.sync.dma_start(den0[0:1, cs], rawat[64:65, cs])
                        nc.vector.reciprocal_approx_fast(denr[0:1, cs],
                                                         den0[0:1, cs])
                        nc.gpsimd.partition_broadcast(div[0:64, cs],
                                                      denr[0:1, cs])
                        nc.vector.tensor_mul(
                            attnT_t[hc][hb:hb + 64,
                                        tw * W + half * 512:
                                        tw * W + (half + 1) * 512],
                            rawat[0:64, cs], div[0:64, cs])

                    prev = None
                    for idx, (c, off, w) in enumerate(chunks):
                        sp = escore(idx, c, off, w)
                        if prev is not None:
                            eav(*prev)
                            if split_nrm and prev[0] == last[0]:
                                nrm_half(0)
                        prev = (idx, c, off, w, sp)
                        if nrm_q:
                            nrm_q.pop(0)()
                        n = -(-len(fillers) // max(chunks_left[0], 1))
                        for _ in range(min(n, 4)):
                            if fillers:
                                fillers.pop(0)()
                        chunks_left[0] -= 1
                    eav(*prev)
                    if split_nrm:
                        nrm_half(1)
                        return

                    # normalize: PSUM-releasing copy now; the latency-laden
                    # dma+recip+bcast+mul chain is deferred into the next
                    # attention stretch so it never blocks the DVE queue.
                    nc.vector.tensor_copy(rawat[0:65, :], at[0:65, :])

                    def n_dma():
                        nc.sync.dma_start(den0[0:1, :], rawat[64:65, :])

                    def n_recip():
                        nc.vector.reciprocal_approx_fast(denr[0:1, :],
                                                         den0[0:1, :])

                    def n_bcast():
                        nc.gpsimd.partition_broadcast(div[0:64, :],
                                                      denr[0:1, :])

                    def n_mul():
                        nc.vector.tensor_mul(
                            attnT_t[hc][hb:hb + 64, tw * W:(tw + 1) * W],
                            rawat[0:64, :], div[0:64, :])
                    defer_to.extend([n_dma, n_recip, n_bcast, n_mul])

                wo_t = [None] * 4

                def oproj_unit(sc):
                    cbs = []
                    st = {}

                    def alloc():
                        st["ot"] = otp.tile([128, HID], F32, tag="ot",
                                            bufs=2, name="ot")
                    cbs.append(alloc)
                    for n in range(2):
                        def palloc(n=n):
                            st["po"] = pop.tile([128, 512], F32, tag="po",
                                                bufs=2, name="po")
                        cbs.append(palloc)
                        for kc in range(4):
                            def mm(n=n, kc=kc):
                                nc.tensor.matmul(
                                    st["po"][:, :],
                                    attnT_t[kc][:, sc * 128:(sc + 1) * 128],
                                    wo_t[kc][:, n * 512:(n + 1) * 512],
                                    start=(kc == 0), stop=(kc == 3))
                            cbs.append(mm)

                        def drain(n=n):
                            nc.vector.tensor_copy(
                                st["ot"][:, n * 512:(n + 1) * 512],
                                st["po"][:, :])
                        cbs.append(drain)

                    def store():
                        nc.sync.dma_start(
                            out_d[sc * 128:(sc + 1) * 128, :], st["ot"][:, :])
                    cbs.append(store)
                    return cbs

                # window 0: V1 projections (sc 8..15) fill the gaps
                v1_pool_ref[0] = (pop, "po")
                fillers = []
                for cbs in v1_units:
                    fillers.extend(cbs)
                left = [8 * HPG]
                nrm_q = []
                w0_tail_nrm = []
                for h in range(HPG):
                    defer = nrm_q if h < HPG - 1 else w0_tail_nrm
                    attn_head(h, 0, fillers, nrm_q, left, defer)
                while nrm_q:
                    nrm_q.pop(0)()
                while fillers:
                    fillers.pop(0)()

                # window 1: w0 output projections fill the gaps
                fillers = []
                def ld_wo():
                    for kc in range(4):
                        wo = wop.tile([128, HID], BF16, tag=f"wo{kc}",
                                      name=f"wo{kc}")
                        nc.sync.dma_start(
                            wo[:, :], woT_d[kc * 128:(kc + 1) * 128, :])
                        wo_t[kc] = wo
                fillers.append(ld_wo)
                for sc in range(8):
                    fillers += oproj_unit(sc)
                left = [16 * HPG]
                nrm_q = list(w0_tail_nrm)
                for h in range(HPG):
                    attn_head(h, 1, fillers, nrm_q, left, nrm_q,
                              split_nrm=(h == HPG - 1))
                while nrm_q:
                    nrm_q.pop(0)()
                while fillers:
                    fillers.pop(0)()
                for sc in range(8, SC):
                    for cb in oproj_unit(sc):
                        cb()

    nc.compile()
    return nc


def kernel(hidden_states, causal_mask, padding_mask,
           q_w, q_b, k_w, k_b, v_w, v_b, o_w, o_b):
    global _compiled
    import ml_dtypes
    from concourse.bass_utils import run_bass_kernel_spmd

    hidden_states = np.asarray(hidden_states, dtype=np.float32)
    padding_mask = np.asarray(padding_mask)
    q_w = np.asarray(q_w, dtype=np.float32)
    k_w = np.asarray(k_w, dtype=np.float32)
    v_w = np.asarray(v_w, dtype=np.float32)
    o_w = np.asarray(o_w, dtype=np.float32)
    q_b = np.asarray(q_b, dtype=np.float32)
    v_b = np.asarray(v_b, dtype=np.float32)
    o_b = np.asarray(o_b, dtype=np.float32)

    if _compiled is None:
        _compiled = _build()
    nc = _compiled

    in_maps = []
    for b in range(BS):
        xT = np.ascontiguousarray(hidden_states[b].T)
        xTb = xT.astype(ml_dtypes.bfloat16)
        kmask = np.where(padding_mask[b], np.float32(NEG),
                         np.float32(0.0)).astype(np.float32)
        kmask2 = np.ascontiguousarray(kmask.reshape(SC, 128).T)
        for g in range(HG):
            r = slice(g * OG, (g + 1) * OG)
            in_maps.append({
                "xT": xT,
                "xTb": xTb,
                "wqT": np.ascontiguousarray(q_w[r].T),
                "wkT": np.ascontiguousarray(k_w[r].T),
                "wvT": np.ascontiguousarray(v_w[r].T).astype(
                    ml_dtypes.bfloat16),
                "woT": np.ascontiguousarray(o_w[:, r].T).astype(
                    ml_dtypes.bfloat16),
                "qb": np.ascontiguousarray(q_b[r].reshape(4, 128).T),
                "kmask": kmask2,
            })

    trace = os.environ.get("KERNEL_TRACE") == "1"
    res = run_bass_kernel_spmd(nc, in_maps, core_ids=list(range(NCORES)),
                               trace=trace)
    if trace and res.exec_time_ns is not None:
        print(f"HW exec time: {res.exec_time_ns} ns")
        if res.instructions_and_trace:
            print(f"trace: {res.instructions_and_trace[1]}")

    # host: sum head-group partials, add o_b and the V-bias contribution
    vb_term = o_w @ v_b  # [HID]; exact because attention weights sum to 1
    const = (o_b + vb_term)[None, :]
    out = np.empty((BS, SEQ, HID), dtype=np.float32)
    for b in range(BS):
        out[b] = (res.results[2 * b]["out"] + res.results[2 * b + 1]["out"]
                  + const)
    return out


# revision 10
# speedup vs baseline: 1.1353x; 1.0434x over previous
"""Multi-head attention (bs=4, seq=2048, hidden=1024, 16 heads) on 8 trn2 cores.

Sharding: core = (batch b, head-group g): 4 batches x 2 groups of 8 heads.
Each core computes QKV projections for its head slice, causal+padded softmax
attention, and a partial output projection; the host sums the two partial
outputs per batch and adds o_b (+ the V-bias contribution, constant across
queries because attention weights sum to 1). K-bias is dropped entirely
(softmax shift invariance).

Engine plan (per core):
  TensorE: QK projections (fp32r), V projection (bf16, FWL), scores
    kT.T@qT transposed [k, q] (fp32r), AV with augmented-V (ones column ->
    softmax denominators accumulate in PSUM row 64), output projection bf16.
  ACT: exclusively exp (padding mask as per-partition bias).
  DVE: projection drains (+q bias), causal tri-mask on bf16 et, softmax
    normalize muls, PSUM->SBUF copies.
  Pool: denominator row broadcast. DMA: den-row partition move, all loads.
Attention is software-pipelined per head (scores of chunk c+1 issue before
AV of chunk c so TensorE never idles on ACT); V1 projections fill window-0
attention, w0 output projections fill window-1 attention.
"""
import os
import sys

for _p in ("/opt/trn_rl_repo",):
    if _p not in sys.path:
        sys.path.insert(0, _p)

import numpy as np

HID = 1024
HEADS = 16
D = 64
BS = 4
SEQ = 2048
NCORES = 8
HG = 2             # head groups (tensor-parallel axis)
HPG = HEADS // HG  # 8 heads per core
OG = HPG * D       # 512 projection dims per core
KC = HID // 128    # 8 hidden chunks
SC = SEQ // 128    # 16 seq chunks
W = 1024           # attention query window
SCALE = 1.0 / np.sqrt(D)
NEG = -30000.0

_compiled = None


def _build():
    import concourse.tile as tile
    from concourse import bacc, mybir

    F32 = mybir.dt.float32
    F32R = mybir.dt.float32r
    BF16 = mybir.dt.bfloat16
    AF = mybir.ActivationFunctionType
    Alu = mybir.AluOpType

    nc = bacc.Bacc("TRN2", target_bir_lowering=False, debug=False,
                   num_devices=NCORES)

    xTb_d = nc.dram_tensor("xTb", [HID, SEQ], BF16, kind="ExternalInput").ap()
    wqT_d = nc.dram_tensor("wqT", [HID, OG], BF16, kind="ExternalInput").ap()
    wkT_d = nc.dram_tensor("wkT", [HID, OG], BF16, kind="ExternalInput").ap()
    wvT_d = nc.dram_tensor("wvT", [HID, OG], BF16, kind="ExternalInput").ap()
    woT_d = nc.dram_tensor("woT", [OG, HID], BF16, kind="ExternalInput").ap()
    qb_d = nc.dram_tensor("qb", [128, 4], F32, kind="ExternalInput").ap()
    kmask_d = nc.dram_tensor("kmask", [128, SC], F32, kind="ExternalInput").ap()
    out_d = nc.dram_tensor("out", [SEQ, HID], F32, kind="ExternalOutput").ap()

    with tile.TileContext(nc) as tc:
        with tc.tile_pool(name="const", bufs=1) as cp, \
             tc.tile_pool(name="qT", bufs=1) as qTp, \
             tc.tile_pool(name="kT", bufs=1) as kTp, \
             tc.tile_pool(name="v", bufs=1) as vp, \
             tc.tile_pool(name="attnT", bufs=1) as aTp, \
             tc.tile_pool(name="wv", bufs=1) as wvp, \
             tc.tile_pool(name="xv", bufs=1) as xvp:

            # ---------------- constants ----------------
            ones_f = cp.tile([128, 128], F32, tag="ones_f", name="ones_f")
            nc.gpsimd.memset(ones_f[:, :], 1.0)
            # tri01[p, j] = 1 if j >= p else 0  (keep keys <= query)
            tri01_f = cp.tile([128, 128], F32, tag="tri01_f", name="tri01_f")
            nc.gpsimd.affine_select(tri01_f[:, :], ones_f[:, :],
                                    pattern=[[1, 128]],
                                    compare_op=Alu.is_ge, fill=0.0,
                                    base=0, channel_multiplier=-1)
            tri01 = cp.tile([128, 128], BF16, tag="tri01", name="tri01")
            nc.scalar.copy(tri01[:, :], tri01_f[:, :])
            qb_s = cp.tile([128, 4], F32, tag="qb", name="qb_s")
            nc.sync.dma_start(qb_s[:, :], qb_d[:, :])
            kmask_s = cp.tile([128, SC], F32, tag="km", name="kmask_s")
            nc.sync.dma_start(kmask_s[:, :], kmask_d[:, :])
            # pre-warm the Q7 library for partition_broadcast (first use
            # otherwise costs a ~7us LIBRARY_RELOAD mid-attention)
            bwarm = cp.tile([64, 128], F32, tag="bwarm", name="bwarm")
            nc.gpsimd.partition_broadcast(bwarm[0:64, :], ones_f[0:1, :])

            # ---------------- persistent tensors ----------------
            qT_t = [qTp.tile([128, SEQ], BF16, tag=f"qT{i}", name=f"qT{i}")
                    for i in range(4)]
            kT_t = [kTp.tile([128, SEQ], BF16, tag=f"kT{i}", name=f"kT{i}")
                    for i in range(4)]
            v_t = [vp.tile([128, HPG * 65], BF16, tag=f"v{i}", name=f"v{i}")
                   for i in range(SC)]
            for i in range(SC):
                vv = v_t[i].rearrange("p (h c) -> p h c", c=65)
                nc.gpsimd.memset(vv[:, :, 64:65], 1.0)
            attnT_t = [aTp.tile([128, SEQ], BF16, tag=f"aT{i}", name=f"aT{i}")
                       for i in range(4)]

            # =========== region 1: QK projections (all seq) + V ===========
            with tc.tile_pool(name="wqk", bufs=1) as wp, \
                 tc.tile_pool(name="x", bufs=1) as xp, \
                 tc.tile_pool(name="phA", bufs=1, space="PSUM") as phA:

                wq_t, wk_t, wv_t = [], [], []
                for kc in range(KC):
                    wq = wp.tile([128, OG], BF16, tag=f"wq{kc}",
                                 name=f"wq{kc}")
                    nc.sync.dma_start(wq[:, :],
                                      wqT_d[kc * 128:(kc + 1) * 128, :])
                    wq_t.append(wq)
                for kc in range(KC):
                    wk = wp.tile([128, OG], BF16, tag=f"wk{kc}",
                                 name=f"wk{kc}")
                    nc.sync.dma_start(wk[:, :],
                                      wkT_d[kc * 128:(kc + 1) * 128, :])
                    wk_t.append(wk)
                for kc in range(KC):
                    wv = wvp.tile([128, OG], BF16, tag=f"wv{kc}",
                                  name=f"wv{kc}")
                    nc.sync.dma_start(wv[:, :],
                                      wvT_d[kc * 128:(kc + 1) * 128, :])
                    wv_t.append(wv)

                def x_half_load(half):
                    tiles = []
                    for kc in range(KC):
                        xt = xp.tile([128, W], BF16, tag=f"x{kc}", bufs=2,
                                     name=f"x{half}{kc}")
                        nc.sync.dma_start(
                            xt[:, :], xTb_d[kc * 128:(kc + 1) * 128,
                                            half * W:(half + 1) * W])
                        tiles.append(xt)
                    return tiles

                def qk_unit(wt, oc, xg, half, is_q):
                    p0 = phA.tile([128, 512], F32, tag="pts", bufs=6,
                                  name="p0")
                    p1 = phA.tile([128, 512], F32, tag="pts", bufs=6,
                                  name="p1")
                    for kc in range(KC):
                        for t, pt in ((0, p0), (1, p1)):
                            nc.tensor.matmul(
                                pt[:, :],
                                wt[kc][:, oc * 128:(oc + 1) * 128],
                                xg[kc][:, t * 512:(t + 1) * 512],
                                start=(kc == 0), stop=(kc == KC - 1))
                    o_t = qT_t if is_q else kT_t
                    for t, pt in ((0, p0), (1, p1)):
                        cols = slice(half * W + t * 512,
                                     half * W + t * 512 + 512)
                        if is_q:
                            nc.vector.tensor_scalar(
                                o_t[oc][:, cols], pt[:, :],
                                qb_s[:, oc:oc + 1], None, Alu.add)
                        else:
                            nc.vector.tensor_copy(o_t[oc][:, cols], pt[:, :])

                def v_unit(sc, pool_ref):
                    """V projection for seq chunk sc; bf16 x streamed.
                    pool_ref: 1-elem list holding (psum_pool, tag) at
                    emission time."""
                    cbs = []
                    st = {"xv": [None] * KC}
                    for kc in range(KC):
                        def dma(kc=kc):
                            xv = xvp.tile([128, 128], BF16, tag=f"xv{kc}",
                                          bufs=2, name=f"xv{kc}")
                            nc.sync.dma_start(
                                xv[:, :], xTb_d[kc * 128:(kc + 1) * 128,
                                                sc * 128:(sc + 1) * 128])
                            st["xv"][kc] = xv
                        cbs.append(dma)

                    def alloc():
                        pool, tag = pool_ref[0]
                        st["pv"] = pool.tile([128, 512], F32, tag=tag,
                                             bufs=2, name="pv")
                    cbs.append(alloc)
                    for kc in range(KC):
                        def mm(kc=kc):
                            nc.tensor.matmul(st["pv"][:, :],
                                             st["xv"][kc][:, :],
                                             wv_t[kc][:, :],
                                             start=(kc == 0),
                                             stop=(kc == KC - 1))
                        cbs.append(mm)

                    def drain():
                        src = st["pv"].rearrange("p (h c) -> p h c", c=64)
                        dst = v_t[sc].rearrange("p (h c) -> p h c", c=65)
                        nc.vector.tensor_copy(dst[:, :, 0:64], src[:, :, :])
                    cbs.append(drain)
                    return cbs

                # V first: needs only ~1.5 MB of DMA, hides the cold start
                # while wq/wk/x stream in.
                xg0 = x_half_load(0)
                for sc in range(6):
                    for cb in v_unit(sc, [(phA, "pv")]):
                        cb()
                xg1 = x_half_load(1)
                for oc in range(4):
                    qk_unit(wq_t, oc, xg0, 0, True)
                for oc in range(4):
                    qk_unit(wk_t, oc, xg0, 0, False)
                for sc in range(6, 8):
                    for cb in v_unit(sc, [(phA, "pv")]):
                        cb()
                for oc in range(4):
                    qk_unit(wq_t, oc, xg1, 1, True)
                for oc in range(4):
                    qk_unit(wk_t, oc, xg1, 1, False)

                v1_pool_ref = [None]
                v1_units = [v_unit(sc, v1_pool_ref) for sc in range(8, SC)]

            # ============ region 2: attention + output projection ============
            with tc.tile_pool(name="et", bufs=1) as etp, \
                 tc.tile_pool(name="raw", bufs=1) as rawp, \
                 tc.tile_pool(name="nrm", bufs=1) as nrmp, \
                 tc.tile_pool(name="wo", bufs=1) as wop, \
                 tc.tile_pool(name="ot", bufs=1) as otp, \
                 tc.tile_pool(name="sp", bufs=1, space="PSUM") as spp, \
                 tc.tile_pool(name="at", bufs=1, space="PSUM") as atp, \
                 tc.tile_pool(name="po", bufs=1, space="PSUM") as pop:

                den0 = nrmp.tile([1, W], F32, tag="den0", name="den0")
                denr = nrmp.tile([1, W], F32, tag="denr", name="denr")
                div = nrmp.tile([64, W], F32, tag="div", name="div")

                def attn_head(h, tw, fillers, nrm_q, chunks_left, defer_to,
                              split_nrm=False):
                    hb = (h % 2) * 64
                    hc = h // 2
                    chunks = [(c, 0, W) for c in range(8 * tw)]
                    chunks += [(8 * tw + i, 128 * i, W - 128 * i)
                               for i in range(8)]
                    first = [None, None]
                    last = [None, None]
                    for idx, (c, off, w) in enumerate(chunks):
                        for half in range(2):
                            if max(off, half * 512) < (half + 1) * 512:
                                if first[half] is None:
                                    first[half] = idx
                                last[half] = idx
                    at = atp.tile([65, W], F32, tag="at", bufs=1, name="at")

                    def escore(idx, c, off, w):
                        sp = spp.tile([128, W], F32, tag="sp", bufs=2,
                                      name="sp")
                        for half in range(2):
                            lo = max(off, half * 512)
                            hi = (half + 1) * 512
                            if lo >= hi:
                                continue
                            nc.tensor.matmul(
                                sp[:, lo:hi],
                                kT_t[hc][hb:hb + 64, c * 128:(c + 1) * 128],
                                qT_t[hc][hb:hb + 64,
                                         tw * W + lo:tw * W + hi],
                                start=True, stop=True)
                        return sp

                    def eav(idx, c, off, w, sp):
                        et = etp.tile([128, W], BF16, tag="et", bufs=3,
                                      name="et")
                        nc.scalar.activation(et[:, :w], sp[:, off:off + w],
                                             AF.Exp,
                                             bias=kmask_s[:, c:c + 1],
                                             scale=SCALE)
                        if off or c == 8 * tw:  # diagonal chunk
                            nc.vector.tensor_mul(et[:, 0:128], et[:, 0:128],
                                                 tri01[:, :])
                        for half in range(2):
                            lo = max(off, half * 512)
                            hi = (half + 1) * 512
                            if lo >= hi:
                                continue
                            nc.tensor.matmul(
                                at[0:65, lo:hi],
                                v_t[c][:, h * 65:(h + 1) * 65],
                                et[:, lo - off:hi - off],
                                start=(idx == first[half]),
                                stop=(idx == last[half]))

                    rawat = rawp.tile([65, W], F32, tag="raw", bufs=2,
                                      name="raw")

                    def nrm_half(half):
                        cs = slice(half * 512, (half + 1) * 512)
                        nc.vector.tensor_copy(rawat[0:65, cs], at[0:65, cs])
                        nc.sync.dma_start(den0[0:1, cs], rawat[64:65, cs])
                        nc.vector.reciprocal_approx_fast(denr[0:1, cs],
                                                         den0[0:1, cs])
                        nc.gpsimd.partition_broadcast(div[0:64, cs],
                                                      denr[0:1, cs])
                        nc.vector.tensor_mul(
                            attnT_t[hc][hb:hb + 64,
                                        tw * W + half * 512:
                                        tw * W + (half + 1) * 512],
                            rawat[0:64, cs], div[0:64, cs])

                    prev = None
                    for idx, (c, off, w) in enumerate(chunks):
                        sp = escore(idx, c, off, w)
                        if prev is not None:
                            eav(*prev)
                            if split_nrm and prev[0] == last[0]:
                                nrm_half(0)
                        prev = (idx, c, off, w, sp)
                        if nrm_q:
                            nrm_q.pop(0)()
                        n = -(-len(fillers) // max(chunks_left[0], 1))
                        for _ in range(min(n, 4)):
                            if fillers:
                                fillers.pop(0)()
                        chunks_left[0] -= 1
                    eav(*prev)
                    if split_nrm:
                        nrm_half(1)
                        return

                    # normalize: PSUM-releasing copy now; the latency-laden
                    # dma+recip+bcast+mul chain is deferred into the next
                    # attention stretch so it never blocks the DVE queue.
                    nc.vector.tensor_copy(rawat[0:65, :], at[0:65, :])

                    def n_dma():
                        nc.sync.dma_start(den0[0:1, :], rawat[64:65, :])

                    def n_recip():
                        nc.vector.reciprocal_approx_fast(denr[0:1, :],
                                                         den0[0:1, :])

                    def n_bcast():
                        nc.gpsimd.partition_broadcast(div[0:64, :],
                                                      denr[0:1, :])

                    def n_mul():
                        nc.vector.tensor_mul(
                            attnT_t[hc][hb:hb + 64, tw * W:(tw + 1) * W],
                            rawat[0:64, :], div[0:64, :])
                    defer_to.extend([n_dma, n_recip, n_bcast, n_mul])

                wo_t = [None] * 4

                def oproj_unit(sc):
                    cbs = []
                    st = {}

                    def alloc():
                        st["ot"] = otp.tile([128, HID], F32, tag="ot",
                                            bufs=2, name="ot")
                    cbs.append(alloc)
                    for n in range(2):
                        def palloc(n=n):
                            st["po"] = pop.tile([128, 512], F32, tag="po",
                                                bufs=2, name="po")
                        cbs.append(palloc)
                        for kc in range(4):
                            def mm(n=n, kc=kc):
                                nc.tensor.matmul(
                                    st["po"][:, :],
                                    attnT_t[kc][:, sc * 128:(sc + 1) * 128],
                                    wo_t[kc][:, n * 512:(n + 1) * 512],
                                    start=(kc == 0), stop=(kc == 3))
                            cbs.append(mm)

                        def drain(n=n):
                            nc.vector.tensor_copy(
                                st["ot"][:, n * 512:(n + 1) * 512],
                                st["po"][:, :])
                        cbs.append(drain)

                    def store():
                        nc.sync.dma_start(
                            out_d[sc * 128:(sc + 1) * 128, :], st["ot"][:, :])
                    cbs.append(store)
                    return cbs

                # window 0: V1 projections (sc 8..15) fill the gaps
                v1_pool_ref[0] = (pop, "po")
                fillers = []
                for cbs in v1_units:
                    fillers.extend(cbs)
                left = [8 * HPG]
                nrm_q = []
                w0_tail_nrm = []
                for h in range(HPG):
                    defer = nrm_q if h < HPG - 1 else w0_tail_nrm
                    attn_head(h, 0, fillers, nrm_q, left, defer)
                while nrm_q:
                    nrm_q.pop(0)()
                while fillers:
                    fillers.pop(0)()

                # window 1: w0 output projections fill the gaps
                fillers = []
                def ld_wo():
                    for kc in range(4):
                        wo = wop.tile([128, HID], BF16, tag=f"wo{kc}",
                                      name=f"wo{kc}")
                        nc.sync.dma_start(
                            wo[:, :], woT_d[kc * 128:(kc + 1) * 128, :])
                        wo_t[kc] = wo
                fillers.append(ld_wo)
                for sc in range(8):
                    fillers += oproj_unit(sc)
                left = [16 * HPG]
                nrm_q = list(w0_tail_nrm)
                for h in range(HPG):
                    attn_head(h, 1, fillers, nrm_q, left, nrm_q,
                              split_nrm=(h == HPG - 1))
                while nrm_q:
                    nrm_q.pop(0)()
                while fillers:
                    fillers.pop(0)()
                for sc in range(8, SC):
                    for cb in oproj_unit(sc):
                        cb()

    nc.compile()
    return nc


def kernel(hidden_states, causal_mask, padding_mask,
           q_w, q_b, k_w, k_b, v_w, v_b, o_w, o_b):
    global _compiled
    import ml_dtypes
    from concourse.bass_utils import run_bass_kernel_spmd

    hidden_states = np.asarray(hidden_states, dtype=np.float32)
    padding_mask = np.asarray(padding_mask)
    q_w = np.asarray(q_w, dtype=np.float32)
    k_w = np.asarray(k_w, dtype=np.float32)
    v_w = np.asarray(v_w, dtype=np.float32)
    o_w = np.asarray(o_w, dtype=np.float32)
    q_b = np.asarray(q_b, dtype=np.float32)
    v_b = np.asarray(v_b, dtype=np.float32)
    o_b = np.asarray(o_b, dtype=np.float32)

    if _compiled is None:
        _compiled = _build()
    nc = _compiled

    in_maps = []
    for b in range(BS):
        xTb = np.ascontiguousarray(hidden_states[b].T).astype(
            ml_dtypes.bfloat16)
        kmask = np.where(padding_mask[b], np.float32(NEG),
                         np.float32(0.0)).astype(np.float32)
        kmask2 = np.ascontiguousarray(kmask.reshape(SC, 128).T)
        for g in range(HG):
            r = slice(g * OG, (g + 1) * OG)
            in_maps.append({
                "xTb": xTb,
                "wqT": np.ascontiguousarray(q_w[r].T).astype(
                    ml_dtypes.bfloat16),
                "wkT": np.ascontiguousarray(k_w[r].T).astype(
                    ml_dtypes.bfloat16),
                "wvT": np.ascontiguousarray(v_w[r].T).astype(
                    ml_dtypes.bfloat16),
                "woT": np.ascontiguousarray(o_w[:, r].T).astype(
                    ml_dtypes.bfloat16),
                "qb": np.ascontiguousarray(q_b[r].reshape(4, 128).T),
                "kmask": kmask2,
            })

    trace = os.environ.get("KERNEL_TRACE") == "1"
    res = run_bass_kernel_spmd(nc, in_maps, core_ids=list(range(NCORES)),
                               trace=trace)
    if trace and res.exec_time_ns is not None:
        print(f"HW exec time: {res.exec_time_ns} ns")
        if res.instructions_and_trace:
            print(f"trace: {res.instructions_and_trace[1]}")

    # host: sum head-group partials, add o_b and the V-bias contribution
    vb_term = o_w @ v_b  # [HID]; exact because attention weights sum to 1
    const = (o_b + vb_term)[None, :]
    out = np.empty((BS, SEQ, HID), dtype=np.float32)
    for b in range(BS):
        out[b] = (res.results[2 * b]["out"] + res.results[2 * b + 1]["out"]
                  + const)
    return out
